# revision 8
# baseline (speedup 1.0000x reference)
"""Trainium2 Bass kernel for an attention block (B=8, H=W=32, C=256, 4 heads).

Sharding: data-parallel over batch - one batch element per NeuronCore (8 cores).

All matmuls run in fp8e4 with DoubleRow perf mode (K=256 per instruction,
2x PE throughput vs bf16). Scales are folded so every PSUM evacuation is a
plain cast:
  m8 = (Wq_h @ Wk_h^T)*16 per head (host-side: S = x M x^T fuses the q and k
  projections into one), wv8 = Wv*16, wo8 = Wout*32.
  z8 = psum(x8 @ m8) = 16*(x M);  v8 = 16*v
  S_psum = z8 . x8 = 256*S_true -> exp via ACTIVATE(scale=1/256, bias=-3.5)
  expS8 (fp8, unnormalized; the e^-3.5 factor cancels in the softmax ratio)
  denom via DoubleRow ones-matmul: dps = sum_j expS8 -> recip -> rbc = 16/dps
  O_psum = v8^T @ expS8 = 16*unnorm-O ; ocT8 = O_psum * rbc = 16*O_true
  res_psum = ocT8 @ wo8 = 512*res ; evac: res = psum/512 + (x+b)

Schedule highlights (see the inline comments):
  - ~8us DMA-engine startup latency + 2.4GHz HAM clock ramp are covered by
    dummy warm-up matmuls; gpsimd (slow ~7us engine start) gets no early work.
  - attention is a depth-2 software pipeline over (head, i-half) steps:
    S two steps ahead of denom+O, so the PE never waits on ScalarE's exp.
  - i-halves are processed outer so each half's out-projection (+residual,
    +store) overlaps the other half's attention.
  - evacuations are balanced: exp on scalar, projection/O evacs split
    scalar/vector, residual adds on gpsimd, stores on both hardware DMA
    queues.
"""

import numpy as np
import ml_dtypes

import concourse.bass as bass
import concourse.tile as tile
from concourse import bacc, mybir
from concourse import bass_utils
from concourse.masks import make_identity

P = 128
N_TOK = 1024          # tokens per batch element (32*32)
C = 256               # channels
NH = 4                # heads
HD = 256              # head dim (= C)
NT = N_TOK // P       # 8 token tiles
NC = C // P           # 2 channel chunks
F32 = mybir.dt.float32
F32R = mybir.dt.float32r
BF16 = mybir.dt.bfloat16
FP8 = mybir.dt.float8e4
DR = mybir.MatmulPerfMode.DoubleRow

S_QK = 16.0           # q8,k8 = 16*(q,k)
S_O = 16.0            # ocT8 = 16*O
S_W = 32.0            # wo8 = 32*Wout
EXP_SHIFT = -3.5      # exp(S - 3.5): max |S| ~ 7.5 -> expS8 <= ~55 (fp8 max 240),
                      # and large softmax weights land in a better fp8 binade


def _build_program():
    nc = bacc.Bacc("TRN2", target_bir_lowering=False, debug=False, num_devices=8)

    x_d = nc.dram_tensor("x", [N_TOK, C], F32, kind="ExternalInput").ap()
    xbf_d = nc.dram_tensor("xbf", [N_TOK, C], BF16, kind="ExternalInput").ap()
    m_d = nc.dram_tensor("m", [C, NH * HD], FP8, kind="ExternalInput").ap()
    wv_d = nc.dram_tensor("wv", [C, NH * HD], FP8, kind="ExternalInput").ap()
    wo_d = nc.dram_tensor("wo", [NH * HD, C], FP8, kind="ExternalInput").ap()
    bo_d = nc.dram_tensor("bo", [1, C], F32R, kind="ExternalInput").ap()
    y_d = nc.dram_tensor("y", [N_TOK, C], F32, kind="ExternalOutput").ap()
    y_r = y_d.rearrange("(t p) c -> p t c", p=P)

    with tile.TileContext(nc) as tc:
        with (
            tc.tile_pool(name="singles", bufs=1) as singles,
            tc.tile_pool(name="spool", bufs=4) as spool,       # expSt tiles
            tc.tile_pool(name="rpool", bufs=3) as rpool,       # rbc tiles
            tc.tile_pool(name="respool", bufs=4) as respool,
            tc.tile_pool(name="ps_big", bufs=3, space="PSUM") as ps_big,     # 6 banks
            tc.tile_pool(name="ps_small", bufs=2, space="PSUM") as ps_small, # 2 banks
        ):
            # ---- constants -------------------------------------------------
            # warm-up operands come from VECTOR memsets: the gpsimd engine
            # takes ~7us to execute its first instruction, the DVE does not.
            warmw = singles.tile([P, P], BF16)
            id_bf = singles.tile([P, P], BF16)
            dummy = singles.tile([P, 512], BF16)
            ones8 = singles.tile([P, 2, P], FP8)
            ones_f = singles.tile([1, P], F32)
            nc.vector.memset(ones_f[:], 1.0)
            ones_sb = singles.tile([1, P], F32R)
            nc.vector.tensor_copy(ones_sb[:], ones_f[:])
            ebias = singles.tile([P, 1], F32)
            nc.vector.memset(ebias[:], EXP_SHIFT)

            # ---- input DMAs (2 hardware queues; gpsimd stays DMA-free so
            # the identity/dummy tiles are ready for the PE warm-up) ---------
            x_r = x_d.rearrange("(t p) c -> p t c", p=P)
            xbf_r = xbf_d.rearrange("(t p) c -> p t c", p=P)
            bo_sb = singles.tile([1, C], F32R)
            nc.sync.dma_start(bo_sb[:], bo_d[:])
            # bf16 copy of x feeds the compute path (arrives in half the
            # time); the f32 x is only needed for the residual add (~50us in)
            # so it loads last.
            xbf = singles.tile([P, NT, C], BF16)
            nc.sync.dma_start(xbf[:], xbf_r[:])
            m8 = singles.tile([P, NC, NH * HD], FP8)
            nc.scalar.dma_start(m8[:], m_d.rearrange("(k p) d -> p k d", p=P))
            wv8 = singles.tile([P, NC, NH * HD], FP8)
            nc.scalar.dma_start(wv8[:], wv_d.rearrange("(k p) d -> p k d", p=P))
            wo8 = singles.tile([P, NT, C], FP8)
            nc.scalar.dma_start(wo8[:], wo_d.rearrange("(k p) c -> p k c", p=P))
            xsb = singles.tile([P, NT, C], F32)
            nc.sync.dma_start(xsb[:, :NT // 2, :], x_r[:, :NT // 2, :])
            nc.scalar.dma_start(xsb[:, NT // 2:, :], x_r[:, NT // 2:, :])

            nc.vector.memset(warmw[:], 0.5)
            nc.vector.memset(dummy[:], 0.5)
            make_identity(nc, id_bf[:])
            nc.gpsimd.memset(ones8[:], 1.0)

            # ---- HAM warm-up: dummy matmuls while the input DMAs land ------
            # (PE clock defaults to 1.2 GHz; sustained activity flips it to
            # 2.4 GHz. The DMA engines have ~8us of startup latency, so there
            # is nothing real for the PE to do before ~11us anyway.)
            def warm(n):
                for r in range(n):
                    pw = ps_small.tile([P, 512], F32, tag="pss")
                    nc.tensor.matmul(pw[:], warmw[:], dummy[:],
                                     start=True, stop=True)

            warm(20)

            # ---- x^T (fp8) via PE transposes (bf16 in, cast at evacuation) -
            xT8 = singles.tile([P, NC, N_TOK], FP8)
            for th in range(2):
                for cc in range(NC):
                    pst = ps_small.tile([P, 512], BF16, tag="pss")
                    for k in range(4):
                        t = th * 4 + k
                        nc.tensor.transpose(
                            pst[:, k * P:(k + 1) * P],
                            xbf[:, t, cc * P:(cc + 1) * P],
                            id_bf[:],
                        )
                    nc.scalar.copy(
                        xT8[:, cc, th * 512:(th + 1) * 512], pst[:]
                    )

            # ---- xpb = x + b (gpsimd) --------------------------------------
            xpb = singles.tile([P, NT, C], F32)
            psb = ps_small.tile([P, C], F32, tag="pss")
            nc.tensor.matmul(psb[:], ones_sb[:], bo_sb[:], start=True, stop=True)
            bbc = singles.tile([P, C], F32)
            nc.vector.tensor_copy(bbc[:], psb[:])
            for t in range(NT):
                nc.gpsimd.tensor_add(xpb[:, t, :], xsb[:, t, :], bbc[:])

            # ---- projections (all DoubleRow, K=256) ------------------------
            # z^T = (M_h^T x^T): [c'-part, dt, h, i] = 16*(x @ M_h)^T where
            # M_h = Wq_h Wk_h^T is precomputed on the host (S = x M x^T fuses
            # the q and k projections into one).
            zT8 = singles.tile([P, NC, NH, N_TOK], FP8)
            va8 = singles.tile([P, NT, NH * HD], FP8)
            ei = [0]

            def _evac(dst, psq, mix):
                if mix and ei[0] % 2 == 0:
                    nc.scalar.copy(dst, psq[:])
                else:
                    nc.vector.tensor_copy(dst, psq[:])
                ei[0] += 1

            def z_proj(h, mix=True):
                for dt_ in range(NC):
                    d0 = (h * NC + dt_) * P
                    psq = ps_big.tile([P, N_TOK], F32, tag="psb")
                    for ih in range(2):
                        nc.tensor.matmul(
                            psq[:, ih * 512:(ih + 1) * 512],
                            m8[:, :, d0:d0 + P],
                            xT8[:, :, ih * 512:(ih + 1) * 512],
                            start=True, stop=True, perf_mode=DR,
                        )
                    _evac(zT8[:, dt_, h, :], psq, mix)

            def v_proj(t, mix=True):
                psv = ps_big.tile([P, N_TOK], F32, tag="psb")
                for dh in range(2):
                    nc.tensor.matmul(
                        psv[:, dh * 512:(dh + 1) * 512],
                        xT8[:, :, t * P:(t + 1) * P],
                        wv8[:, :, dh * 512:(dh + 1) * 512],
                        start=True, stop=True, perf_mode=DR,
                    )
                _evac(va8[:, t, :], psv, mix)

            # ---- attention: ih outer, heads inner, pipelined ---------------
            ocT8 = singles.tile([P, NT, N_TOK], FP8)   # [hd-part, kc, i] = 16*O^T

            def s_phase(h, ih):
                """S^T + exp for (h, ih): returns expSt tile [128, 8jt, 512]."""
                expSt = spool.tile([P, NT, 512], FP8, tag="expS")
                for jp in range(4):
                    pss = ps_big.tile([P, N_TOK], F32, tag="psb")
                    for u in range(2):
                        jt = 2 * jp + u
                        nc.tensor.matmul(
                            pss[:, u * 512:(u + 1) * 512],
                            xT8[:, :, jt * P:(jt + 1) * P],
                            zT8[:, :, h, ih * 512:(ih + 1) * 512],
                            start=True, stop=True, perf_mode=DR,
                        )
                    nc.scalar.activation(
                        expSt[:, 2 * jp:2 * jp + 2, :], pss[:],
                        mybir.ActivationFunctionType.Exp,
                        bias=ebias[:], scale=1.0 / (S_QK * (C ** 0.5)),
                    )
                return expSt

            def do_phase(h, ih, expSt):
                """denominators + O^T for (h, ih); consumes expSt."""
                dps = ps_small.tile([P, 512], F32, tag="pss")
                for jp in range(4):
                    nc.tensor.matmul(
                        dps[:], ones8[:], expSt[:, 2 * jp:2 * jp + 2, :],
                        start=(jp == 0), stop=(jp == 3), perf_mode=DR,
                    )
                rbc = rpool.tile([P, 512], F32, tag="rbc")
                nc.vector.reciprocal_approx_fast(rbc[:], dps[:])
                for dt_ in range(NC):
                    pso = ps_small.tile([P, 512], F32, tag="pss")
                    for jp in range(4):
                        nc.tensor.matmul(
                            pso[:],
                            va8[:, 2 * jp:2 * jp + 2,
                                (h * NC + dt_) * P:(h * NC + dt_ + 1) * P],
                            expSt[:, 2 * jp:2 * jp + 2, :],
                            start=(jp == 0), stop=(jp == 3), perf_mode=DR,
                        )
                    dst = ocT8[:, h * NC + dt_, ih * 512:(ih + 1) * 512]
                    nc.vector.tensor_mul(dst, pso[:], rbc[:])

            def op_phase(ih):
                """out-proj + residual + store for i-half ih."""
                for it4 in range(4):
                    it = ih * 4 + it4
                    pr = ps_big.tile([P, C], F32, tag="psb")
                    for kp in range(4):
                        nc.tensor.matmul(
                            pr[:],
                            ocT8[:, 2 * kp:2 * kp + 2, it * P:(it + 1) * P],
                            wo8[:, 2 * kp:2 * kp + 2, :],
                            start=(kp == 0), stop=(kp == 3), perf_mode=DR,
                        )
                    res = respool.tile([P, C], F32, tag="res")
                    nc.vector.scalar_tensor_tensor(
                        res[:], pr[:], 1.0 / (S_O * S_W), xpb[:, it, :],
                        op0=mybir.AluOpType.mult, op1=mybir.AluOpType.add,
                    )
                    # ih=0 stores issue from the idle sync queue only: a
                    # DMA_DIRECT2D occupies the issuing engine ~650ns, and
                    # mid-attention the scalar engine is pacing exp. For
                    # ih=1 (exp done) both queues drain the tail faster.
                    if ih == 0 or it % 2 == 0:
                        nc.sync.dma_start(y_r[:, it, :], res[:])
                    else:
                        nc.scalar.dma_start(y_r[:, it, :], res[:])

            # Projections for heads 0/1 + half of v first (inputs only land at
            # ~15us due to DMA startup latency; evacs split scalar/vector).
            # The rest of the projections interleave into the first attention
            # steps with their evacuations on VECTOR (scalar is saturated by
            # exp once attention starts). Attention runs as a depth-2
            # software pipeline: S two steps ahead of denom+O.
            warm(2)   # bridge the m8-DMA wait, keep HAM up
            for h in range(NH):
                z_proj(h)
            warm(2)   # bridge any remaining wv-DMA wait
            for t in range(NT):
                v_proj(t)
            steps = [(h, ih) for ih in range(2) for h in range(NH)]
            pend = []
            for si, (h, ih) in enumerate(steps):
                pend.append((h, ih, s_phase(h, ih)))
                depth = 2
                while len(pend) > depth:
                    ph, pih, pexp = pend.pop(0)
                    do_phase(ph, pih, pexp)
                    if ph == NH - 1:
                        op_phase(pih)
            for ph, pih, pexp in pend:
                do_phase(ph, pih, pexp)
                if ph == NH - 1:
                    op_phase(pih)

    nc.compile()
    return nc


_NC_CACHE = {}


def _get_program():
    if "nc" not in _NC_CACHE:
        _NC_CACHE["nc"] = _build_program()
    return _NC_CACHE["nc"]


def _fp8(a):
    return np.asarray(a, dtype=np.float32).astype(ml_dtypes.float8_e4m3)


def _make_in_maps(x, W_qkv, W_out, b_out):
    B = x.shape[0]
    x = np.ascontiguousarray(x.reshape(B, N_TOK, C), dtype=np.float32)
    # W_qkv [C, h*3C]: head-major columns; q: slot<C, k: C<=slot<2C, v: rest.
    w = np.asarray(W_qkv, dtype=np.float32).reshape(C, NH, 3 * C)
    # M_h = Wq_h @ Wk_h^T fuses the q/k projections: S = x M x^T.
    m = np.stack([w[:, h, :C] @ w[:, h, C:2 * C].T for h in range(NH)], axis=1)
    m8 = _fp8(m.reshape(C, NH * HD) * S_QK)
    wv = _fp8(w[:, :, 2 * C:].reshape(C, NH * HD) * S_QK)
    wo = _fp8(np.asarray(W_out, dtype=np.float32) * S_W)
    bo = np.ascontiguousarray(np.asarray(b_out, dtype=np.float32).reshape(1, C))
    xbf = x.astype(ml_dtypes.bfloat16)
    return [
        {"x": x[b], "xbf": xbf[b], "m": m8, "wv": wv, "wo": wo, "bo": bo}
        for b in range(B)
    ]


def run_spmd(x, W_qkv, W_out, b_out, **runner_kwargs):
    """Run on the 8 cores; returns (BassKernelResults, assembled output)."""
    nc = _get_program()
    in_maps = _make_in_maps(x, W_qkv, W_out, b_out)
    res = bass_utils.run_bass_kernel_spmd(
        nc, in_maps, core_ids=list(range(8)), **runner_kwargs
    )
    B, H, W = x.shape[0], x.shape[1], x.shape[2]
    y = np.stack([res.results[b]["y"] for b in range(B)])
    return res, y.reshape(B, H, W, C).astype(np.float32)


def kernel(x, W_qkv, W_out, b_out):
    _, y = run_spmd(x, W_qkv, W_out, b_out)
    return y
